# revision 8
# baseline (speedup 1.0000x reference)
"""EntityAttention Trainium2 kernel (nn_EntityAttention_31525059952740), v2.

Math (per batch b -> core b; all 16 entities, 64 events):
  scoresT[s,(h,e)] = toksT.T @ wtil          (wtil = (q*scale) @ Wk folded host-side)
  E = exp(scoresT)                            bf16
  denom[ent,(h,e)] = masksT.T @ E             (PE matmul, masked sum over s)
  attnT[(g,sc)]    = E * maskT (per-entity scalar)   bf16, 4 ents per tile
  V = toks @ WvT                              PSUM -> bf16 SBUF
  po[d,(4e,64)]    = V_sc.T @ attnT slices    (PV, unnormalized)
  outT_u           = copy(po)  -> outT = outT_u * srec_bc   (normalize late,
                     decouples PSUM drain from the srec DRAM-roundtrip latency)
  O[(2e,64),:]     = outT.T @ WoT  -> bf16 out rows
Host applies bo + bv@Wo.T and the ragged gather.

Perf notes (TimelineSim cost model):
 - matmul cost = out free-dim rows; f32r pays 4x under 256 rows -> bf16
   everywhere (also halves DMA bytes; all DMAs contiguous >=512B runs).
 - PE p-state ramps over 3us -> dummy warm-up matmuls at t~0.
 - PSUM slot = full 2KB bank (8 total): tags small(2) + pv/pO(4) + po(2).
"""

import numpy as np

import concourse.bass as bass
import concourse.tile as tile
import concourse.mybir as mybir
from concourse import bacc
from concourse.bass_utils import run_bass_kernel_spmd

NB, SL, NH, EN, NE, HEADS = 8, 512, 512, 16, 64, 2
DH = NH // HEADS          # 256
P = 128
NCHUNK = NH // P          # 4 hidden-dim chunks
SCHUNK = SL // P          # 4 sequence chunks
HE = HEADS * NE           # 128
SCALE = 1.0 / np.sqrt(DH).astype(np.float32)
N_WARM = 14

F32 = mybir.dt.float32
BF16 = mybir.dt.bfloat16
NPBF16 = mybir.dt.np(BF16)

_CACHE = {}


def _build():
    nc = bacc.Bacc("TRN2", target_bir_lowering=False, debug=False, num_devices=NB)

    # ---- I/O (all bf16, host-prepacked so every DMA is contiguous) ----
    # stream0: [toksT_hc0 | wtil_hc0]; streams 1-3: [toksT | wtil | wvT]_hc
    s0_d = nc.dram_tensor("s0", [P, SL + HE], BF16, kind="ExternalInput").ap()
    wv0_d = nc.dram_tensor("wv0", [P, NH], BF16, kind="ExternalInput").ap()
    sk_d = [nc.dram_tensor(f"s{k}", [P, SL + HE + NH], BF16,
                           kind="ExternalInput").ap() for k in (1, 2, 3)]
    masksT_d = nc.dram_tensor("masksT", [P, SCHUNK * EN], BF16,
                              kind="ExternalInput").ap()
    masksF_d = nc.dram_tensor("masksF", [P, SCHUNK * EN], F32,
                              kind="ExternalInput").ap()
    wo_d = nc.dram_tensor("wo", [P, NCHUNK * NH], BF16,
                          kind="ExternalInput").ap()
    out_d = nc.dram_tensor("out", [EN * NE, NH], BF16, kind="ExternalOutput").ap()

    EXP = mybir.ActivationFunctionType.Exp
    CPY = mybir.ActivationFunctionType.Copy

    with tile.TileContext(nc) as tc:
        with (
            tc.tile_pool(name="sb", bufs=1) as sb,
            tc.tile_pool(name="ps", bufs=1, space="PSUM") as ps,
            tc.tile_pool(name="dram", bufs=1, space="DRAM") as dram,
        ):
            # ---------- input DMAs (sync queue, consumption order) --------
            s0_t = sb.tile([P, SL + HE], BF16, tag="s0")
            nc.sync.dma_start(s0_t[:], s0_d)
            wv0_t = sb.tile([P, NH], BF16, tag="wv0")
            nc.sync.dma_start(wv0_t[:], wv0_d)
            sk_t = []
            for k in range(3):
                t = sb.tile([P, SL + HE + NH], BF16, tag=f"sk{k}")
                nc.sync.dma_start(t[:], sk_d[k])
                sk_t.append(t)
            masksT_t = sb.tile([P, SCHUNK, EN], BF16, tag="masksT")
            nc.sync.dma_start(
                masksT_t[:], masksT_d.rearrange("p (c e) -> p c e", c=SCHUNK))
            masksF_t = sb.tile([P, SCHUNK, EN], F32, tag="masksF")
            nc.sync.dma_start(
                masksF_t[:], masksF_d.rearrange("p (c e) -> p c e", c=SCHUNK))
            wo_t = sb.tile([P, NCHUNK, NH], BF16, tag="wo")
            nc.sync.dma_start(
                wo_t[:], wo_d.rearrange("p (c d) -> p c d", c=NCHUNK))

            def toks(hc):
                return (s0_t if hc == 0 else sk_t[hc - 1])[:, 0:SL]

            def wtil(hc):
                t = s0_t if hc == 0 else sk_t[hc - 1]
                return t[:, SL:SL + HE]

            def wvc(hc):
                return wv0_t[:] if hc == 0 else sk_t[hc - 1][:, SL + HE:]

            # ---------- PE warm-up (beat the p-state ramp) ----------------
            scratch = sb.tile([P, 256], BF16, tag="scratch")
            nc.vector.memset(scratch[:], 0)
            warm_ps = ps.tile([P, NH], F32, tag="small", bufs=2, name="warm")
            for i in range(N_WARM):
                nc.tensor.matmul(warm_ps[:, 0:256], scratch[:, 0:128],
                                 scratch[:], start=True, stop=True)

            # ---------- scores + V, streamed per hidden chunk -------------
            pss = ps.tile([P, SCHUNK * HE], F32, tag="small", bufs=2, name="pss")
            pv = [ps.tile([P, NH], F32, tag="pv", bufs=4, name=f"pv{i}")
                  for i in range(SCHUNK)]
            for hc in range(NCHUNK):
                for sc in range(SCHUNK):
                    nc.tensor.matmul(
                        pss[:, sc * HE:(sc + 1) * HE],
                        toks(hc)[:, sc * P:(sc + 1) * P], wtil(hc),
                        start=(hc == 0), stop=(hc == NCHUNK - 1))
                for i in range(SCHUNK):
                    nc.tensor.matmul(
                        pv[i][:], toks(hc)[:, i * P:(i + 1) * P], wvc(hc),
                        start=(hc == 0), stop=(hc == NCHUNK - 1))

            # ---------- exp (Act), V copies + attnT masking ---------------
            e_t = sb.tile([P, SCHUNK, HE], BF16, tag="E")
            for sc in range(SCHUNK):
                nc.scalar.activation(e_t[:, sc, :], pss[:, sc * HE:(sc + 1) * HE],
                                     EXP)

            v = [sb.tile([P, NH], BF16, tag=f"v{i}", name=f"v{i}")
                 for i in range(SCHUNK)]
            attnT = {}
            for g in range(4):
                for sc in range(SCHUNK):
                    attnT[(g, sc)] = sb.tile([P, 4, HE], BF16,
                                             tag=f"attnT{g}_{sc}",
                                             name=f"attnT{g}_{sc}")

            def mask_op(eng, g, sc, k):
                col = 4 * g + k
                if eng is nc.scalar:
                    eng.activation(attnT[(g, sc)][:, k, :], e_t[:, sc, :],
                                   CPY, scale=masksF_t[:, sc, col:col + 1])
                else:
                    eng.tensor_scalar_mul(
                        attnT[(g, sc)][:, k, :], e_t[:, sc, :],
                        masksF_t[:, sc, col:col + 1])

            # g0 masks + V copies interleaved on DVE/Pool (critical for PV-g0)
            nc.vector.tensor_copy(v[0][:], pv[0][:])
            nc.gpsimd.tensor_copy(v[1][:], pv[1][:])
            for sc in range(SCHUNK):
                mask_op(nc.vector, 0, sc, 0)
                mask_op(nc.vector, 0, sc, 1)
                mask_op(nc.gpsimd, 0, sc, 2)
                if sc == 0:
                    nc.vector.tensor_copy(v[2][:], pv[2][:])
                    nc.gpsimd.tensor_copy(v[3][:], pv[3][:])
                mask_op(nc.vector, 0, sc, 3)
            # remaining groups: DVE k0/k1, Pool k2, Act k3
            for g in range(1, 4):
                for sc in range(SCHUNK):
                    mask_op(nc.vector, g, sc, 0)
                    mask_op(nc.vector, g, sc, 1)
                    mask_op(nc.gpsimd, g, sc, 2)
                    mask_op(nc.scalar, g, sc, 3)

            # ---------- denominators + PV (PE), denom interleaved ---------
            pS = ps.tile([EN, HE], F32, tag="small", bufs=2, name="pS")
            po = {}

            def pv_mm(g, sc, dc):
                h = dc // 2
                key = (g, dc)
                if key not in po:
                    po[key] = ps.tile([P, 4 * NE], F32, tag="po", bufs=2,
                                      name=f"po{g}_{dc}")
                nc.tensor.matmul(
                    po[key][:], v[sc][:, dc * P:(dc + 1) * P],
                    attnT[(g, sc)][:, :, h * NE:(h + 1) * NE],
                    start=(sc == 0), stop=(sc == SCHUNK - 1))

            # grp0: dn-sc interleaved, dc01 then dc23 (po ring = 2 banks)
            for sc in range(SCHUNK):
                nc.tensor.matmul(pS[:], masksT_t[:, sc, :], e_t[:, sc, :],
                                 start=(sc == 0), stop=(sc == SCHUNK - 1))
                pv_mm(0, sc, 0)
                pv_mm(0, sc, 1)
            for sc in range(SCHUNK):
                pv_mm(0, sc, 2)
                pv_mm(0, sc, 3)
            for g in range(1, 4):
                for sc in range(SCHUNK):
                    pv_mm(g, sc, 0)
                    pv_mm(g, sc, 1)
                for sc in range(SCHUNK):
                    pv_mm(g, sc, 2)
                    pv_mm(g, sc, 3)

            # ---------- srec: recip + DRAM-roundtrip broadcast ------------
            srec = sb.tile([EN, HE], BF16, tag="srec")
            with nc.allow_low_precision(reason="bf16 softmax denominators"):
                nc.vector.reciprocal(srec[:], pS[:])
            srec_dram = dram.tile([EN, HE], BF16)
            nc.sync.dma_start(srec_dram[:], srec[:])
            srec_bc = []
            for g in range(4):
                t = sb.tile([P, 4, HE], BF16, tag=f"srecbc{g}", name=f"srecbc{g}")
                sd = srec_dram[g * 4:(g + 1) * 4, :]
                nc.sync.dma_start(
                    t[:], bass.AP(tensor=sd.tensor, offset=sd.offset,
                                  ap=[[0, P], *sd.ap]))
                srec_bc.append(t)

            # ---------- po -> unnormalized copy -> normalize --------------
            outT_u = [sb.tile([P, NCHUNK, 4, NE], BF16, tag=f"outTu{g}",
                              name=f"outTu{g}") for g in range(4)]
            outT = [sb.tile([P, NCHUNK, 4, NE], BF16, tag=f"outT{g}",
                            name=f"outT{g}") for g in range(4)]
            # copies in completion order (dc01 first, then dc23) per group
            for g in range(4):
                for dc in range(NCHUNK):
                    eng = nc.vector if dc % 2 == 0 else nc.gpsimd
                    eng.tensor_copy(outT_u[g][:, dc, :, :], po[(g, dc)][:])
            # normalize (bf16 SBUF x bf16 SBUF, DVE 2x mode)
            for g in range(4):
                for dc in range(NCHUNK):
                    h = dc // 2
                    nc.vector.tensor_mul(outT[g][:, dc, :, :],
                                         outT_u[g][:, dc, :, :],
                                         srec_bc[g][:, :, h * NE:(h + 1) * NE])

            # ---------- O projection + output ----------------------------
            for g in range(4):
                for lp in range(2):
                    pair = g * 2 + lp
                    pO = ps.tile([P, NH], F32, tag="pv", bufs=4,
                                 name=f"pO{pair}")
                    for hc in range(NCHUNK):
                        nc.tensor.matmul(
                            pO[:], outT[g][:, hc, 2 * lp:2 * lp + 2, :],
                            wo_t[:, hc, :],
                            start=(hc == 0), stop=(hc == NCHUNK - 1))
                    o_sb = sb.tile([P, NH], BF16, tag=f"osb{pair}",
                                   name=f"osb{pair}")
                    if pair < 7:
                        nc.scalar.activation(o_sb[:], pO[:], CPY)
                    else:
                        # split the last copy for a shorter tail
                        nc.scalar.activation(o_sb[:, :NH // 2], pO[:, :NH // 2],
                                             CPY)
                        nc.vector.tensor_copy(o_sb[:, NH // 2:], pO[:, NH // 2:])
                    nc.sync.dma_start(out_d[pair * P:(pair + 1) * P, :], o_sb[:])

    nc.compile()
    return nc


def _get_nc():
    if "nc" not in _CACHE:
        _CACHE["nc"] = _build()
    return _CACHE["nc"]


def _fast_run(nc, in_maps):
    """Repeat-call path: cached jitted shard_map over the bass PJRT primitive."""
    import jax
    from jax.sharding import Mesh, PartitionSpec
    from jax.experimental.shard_map import shard_map
    import concourse.mybir as mybir_
    from concourse import bass2jax

    if "runner" not in _CACHE:
        bass2jax.install_neuronx_cc_hook()
        part_name = (nc.partition_id_tensor.name
                     if nc.partition_id_tensor else None)
        in_names, out_names, out_avals = [], [], []
        for alloc in nc.m.functions[0].allocations:
            if not isinstance(alloc, mybir_.MemoryLocationSet):
                continue
            name = alloc.memorylocations[0].name
            if alloc.kind == "ExternalInput":
                if name != part_name:
                    in_names.append(name)
            elif alloc.kind == "ExternalOutput":
                out_names.append(name)
                out_avals.append(jax.core.ShapedArray(
                    tuple(alloc.tensor_shape), mybir_.dt.np(alloc.dtype)))
        n_params = len(in_names)
        all_in_names = in_names + out_names
        if part_name is not None:
            all_in_names = all_in_names + [part_name]

        def _body(*args):
            operands = list(args)
            if part_name is not None:
                operands.append(bass2jax.partition_id_tensor())
            outs = bass2jax._bass_exec_p.bind(
                *operands,
                out_avals=tuple(out_avals),
                in_names=tuple(all_in_names),
                out_names=tuple(out_names),
                lowering_input_output_aliases=(),
                sim_require_finite=True,
                sim_require_nnan=True,
                nc=nc,
            )
            return tuple(outs)

        devices = jax.devices()[:NB]
        mesh = Mesh(np.asarray(devices), ("core",))
        n_outs = len(out_names)
        sharded = jax.jit(
            shard_map(_body, mesh=mesh,
                      in_specs=(PartitionSpec("core"),) * (n_params + n_outs),
                      out_specs=(PartitionSpec("core"),) * n_outs,
                      check_rep=False),
            donate_argnums=tuple(range(n_params, n_params + n_outs)),
            keep_unused=True,
        )
        _CACHE["runner"] = (sharded, in_names, out_names, out_avals)

    sharded, in_names, out_names, out_avals = _CACHE["runner"]
    concat_in = [
        np.concatenate([np.asarray(m[name]) for m in in_maps], axis=0)
        for name in in_names
    ]
    concat_zeros = [
        np.zeros((NB * av.shape[0], *av.shape[1:]), av.dtype)
        for av in out_avals
    ]
    out_arrs = sharded(*concat_in, *concat_zeros)
    return [
        {name: np.asarray(out_arrs[i]).reshape(NB, *out_avals[i].shape)[c]
         for i, name in enumerate(out_names)}
        for c in range(NB)
    ]


def kernel(tokens_embed, entities, events_embed, entity_num, entity_masks,
           select_event, Wq, Wk, Wv, bq, bk, bv, Wo, bo):
    tokens_embed = np.asarray(tokens_embed, dtype=np.float32)
    entities = np.asarray(entities)
    events_embed = np.asarray(events_embed, dtype=np.float32)
    entity_masks = np.asarray(entity_masks)
    select_event = np.asarray(select_event)
    Wq = np.asarray(Wq, dtype=np.float32)
    Wk = np.asarray(Wk, dtype=np.float32)
    Wv = np.asarray(Wv, dtype=np.float32)
    Wo = np.asarray(Wo, dtype=np.float32)
    bq = np.asarray(bq, dtype=np.float32)
    bk = np.asarray(bk, dtype=np.float32)
    bv = np.asarray(bv, dtype=np.float32)
    bo = np.asarray(bo, dtype=np.float32)

    nc = _get_nc()

    q_s = (events_embed @ Wq.T + bq) * SCALE          # [NE, NH]
    # fold K projection into the query side (bk cancels in softmax):
    wtil = np.empty((NH, HE), dtype=np.float32)
    for h in range(HEADS):
        hs = slice(h * DH, (h + 1) * DH)
        wtil[:, h * NE:(h + 1) * NE] = (q_s[:, hs] @ Wk[hs, :]).T
    wtil_r = wtil.reshape(NCHUNK, P, HE)
    wv_r = np.ascontiguousarray(Wv.T).reshape(NCHUNK, P, NH)
    wo_r = np.ascontiguousarray(Wo.T).reshape(NCHUNK, P, NH)
    wo_pc = np.ascontiguousarray(
        wo_r.transpose(1, 0, 2).reshape(P, NCHUNK * NH)).astype(NPBF16)
    # attn rows sum to 1 -> bv contributes bv @ Wo.T; applied host-side.
    bo2 = (bo + bv @ Wo.T).astype(np.float32)

    shared = {"wo": wo_pc}
    in_maps = []
    for c in range(NB):
        toks_r = np.ascontiguousarray(tokens_embed[c].T).reshape(NCHUNK, P, SL)
        s0 = np.concatenate([toks_r[0], wtil_r[0]], axis=1).astype(NPBF16)
        streams = {
            "s0": np.ascontiguousarray(s0),
            "wv0": np.ascontiguousarray(wv_r[0]).astype(NPBF16),
        }
        for k in (1, 2, 3):
            sk = np.concatenate([toks_r[k], wtil_r[k], wv_r[k]],
                                axis=1).astype(NPBF16)
            streams[f"s{k}"] = np.ascontiguousarray(sk)
        # masksT[p, sc, ent] = entities[c, ent, sc*128 + p]
        m = entities[c].astype(np.float32)            # [EN, SL]
        mT = m.reshape(EN, SCHUNK, P).transpose(2, 1, 0).reshape(P, -1)
        streams["masksT"] = np.ascontiguousarray(mT).astype(NPBF16)
        streams["masksF"] = np.ascontiguousarray(mT)
        in_maps.append({**streams, **shared})

    if "ran_once" not in _CACHE:
        res = run_bass_kernel_spmd(nc, in_maps, core_ids=list(range(NB)))
        results = res.results
        _CACHE["ran_once"] = True
    else:
        results = _fast_run(nc, in_maps)
    full = np.concatenate(
        [results[c]["out"].astype(np.float32) for c in range(NB)], axis=0)
    full += bo2[None, :]

    # ragged selection (mirrors the reference indexing)
    assert int(entity_num) == EN
    entity_index = np.flatnonzero(entity_masks.reshape(-1))
    pair_sel = (select_event[:, None, :] & entity_masks[:, :, None])
    pair_sel = pair_sel.reshape(-1, NE)[entity_index].reshape(-1)
    event_entity_index = np.flatnonzero(pair_sel)

    sel_rows = (entity_index[:, None] * NE + np.arange(NE)[None, :]).reshape(-1)
    return full[sel_rows][event_entity_index]


# revision 9
# speedup vs baseline: 1.2535x; 1.2535x over previous
"""EntityAttention Trainium2 kernel (nn_EntityAttention_31525059952740), v3.

Per batch b -> core b (16 entities, 64 events):
  scoresT[s,(h,e)] = toksT.T @ wtil       (wtil = (q*scale) @ Wk folded on host)
  E = exp(scoresT) bf16;  denom = masksT.T @ E  (PE);  srec = 1/denom
  attnT = E * maskT (per-entity partition scalar)
  V = toks @ WvT;  po[d,(4e,64)] = V.T @ attnT   (unnormalized PV)
  outT_u = copy(po); outT = outT_u * srec_bc     (normalize once the DRAM
           round-trip broadcast of srec lands; decouples PSUM drain)
  O = outT.T @ WoT -> bf16 rows; host adds bo + bv@Wo.T and gathers.

Cost-model-driven layout: bf16 matmuls (f32r pays 4x under 256 free rows),
contiguous >=512B DMA runs, PE warm-up matmuls against the p-state ramp,
PSUM = 8 banks: warm/scores/pS ring(1) + pv/pO(4) + po dc-pairs(3).
Engine queues are emitted in consumption order; po copies and recip are
kept OFF the mask-heavy queues so the po ring never backs up PV.
"""

import numpy as np

import concourse.bass as bass
import concourse.tile as tile
import concourse.mybir as mybir
from concourse import bacc
from concourse.bass_utils import run_bass_kernel_spmd

NB, SL, NH, EN, NE, HEADS = 8, 512, 512, 16, 64, 2
DH = NH // HEADS          # 256
P = 128
NCHUNK = NH // P          # 4 hidden-dim chunks
SCHUNK = SL // P          # 4 sequence chunks
HE = HEADS * NE           # 128
SCALE = 1.0 / np.sqrt(DH).astype(np.float32)
N_WARM = 10

F32 = mybir.dt.float32
BF16 = mybir.dt.bfloat16
NPBF16 = mybir.dt.np(BF16)

_CACHE = {}


def _build():
    nc = bacc.Bacc("TRN2", target_bir_lowering=False, debug=False, num_devices=NB)

    # ---- I/O (host-prepacked; every DMA contiguous) ----
    s0_d = nc.dram_tensor("s0", [P, SL + HE], BF16, kind="ExternalInput").ap()
    wv0_d = nc.dram_tensor("wv0", [P, NH], BF16, kind="ExternalInput").ap()
    sk_d = [nc.dram_tensor(f"s{k}", [P, SL + HE + NH], BF16,
                           kind="ExternalInput").ap() for k in (1, 2, 3)]
    # masks: cols 0:64 f32 per-(sc,ent) 0/1 scalars; cols 64:96 the same
    # values as bf16 (bitcast view) for the denominator matmul.
    masks_d = nc.dram_tensor("masks", [P, 96], F32, kind="ExternalInput").ap()
    wo_d = nc.dram_tensor("wo", [P, NCHUNK * NH], BF16,
                          kind="ExternalInput").ap()
    out_d = nc.dram_tensor("out", [EN * NE, NH], BF16, kind="ExternalOutput").ap()

    EXP = mybir.ActivationFunctionType.Exp
    CPY = mybir.ActivationFunctionType.Copy

    with tile.TileContext(nc) as tc:
        with (
            tc.tile_pool(name="sb", bufs=1) as sb,
            tc.tile_pool(name="ps", bufs=1, space="PSUM") as ps,
            tc.tile_pool(name="dram", bufs=1, space="DRAM") as dram,
        ):
            # ---------- input DMAs (sync queue, consumption order) --------
            s0_t = sb.tile([P, SL + HE], BF16, tag="s0")
            nc.sync.dma_start(s0_t[:], s0_d)
            wv0_t = sb.tile([P, NH], BF16, tag="wv0")
            nc.sync.dma_start(wv0_t[:], wv0_d)
            sk_t = []
            for k in range(3):
                t = sb.tile([P, SL + HE + NH], BF16, tag=f"sk{k}",
                            name=f"sk{k}")
                nc.sync.dma_start(t[:], sk_d[k])
                sk_t.append(t)
            masks_t = sb.tile([P, 96], F32, tag="masks")
            nc.sync.dma_start(masks_t[:], masks_d)
            wo_t = sb.tile([P, NCHUNK, NH], BF16, tag="wo")
            nc.sync.dma_start(
                wo_t[:], wo_d.rearrange("p (c d) -> p c d", c=NCHUNK))

            masksF = masks_t[:, 0:64].rearrange("p (c e) -> p c e", c=SCHUNK)
            masksB = masks_t[:, 64:96].bitcast(BF16).rearrange(
                "p (c e) -> p c e", c=SCHUNK)

            def toks(hc):
                return (s0_t if hc == 0 else sk_t[hc - 1])[:, 0:SL]

            def wtil(hc):
                t = s0_t if hc == 0 else sk_t[hc - 1]
                return t[:, SL:SL + HE]

            def wvc(hc):
                return wv0_t[:] if hc == 0 else sk_t[hc - 1][:, SL + HE:]

            # ---------- PE warm-up (p-state ramp) -------------------------
            scratch = sb.tile([P, 256], BF16, tag="scratch")
            nc.vector.memset(scratch[:], 0)
            warm_ps = ps.tile([P, 256], F32, tag="small", bufs=1, name="warm")
            for _ in range(N_WARM):
                nc.tensor.matmul(warm_ps[:], scratch[:, 0:128], scratch[:],
                                 start=True, stop=True)

            # ---------- scores + V, streamed per hidden chunk -------------
            pss = ps.tile([P, SCHUNK * HE], F32, tag="small", bufs=1,
                          name="pss")
            pv = [ps.tile([P, NH], F32, tag="pv", bufs=4, name=f"pv{i}")
                  for i in range(SCHUNK)]
            for hc in range(NCHUNK):
                for sc in range(SCHUNK):
                    nc.tensor.matmul(
                        pss[:, sc * HE:(sc + 1) * HE],
                        toks(hc)[:, sc * P:(sc + 1) * P], wtil(hc),
                        start=(hc == 0), stop=(hc == NCHUNK - 1))
                for i in range(SCHUNK):
                    nc.tensor.matmul(
                        pv[i][:], toks(hc)[:, i * P:(i + 1) * P], wvc(hc),
                        start=(hc == 0), stop=(hc == NCHUNK - 1))

            # ---------- exp (2 ops), V copies, masking --------------------
            e_t = sb.tile([P, SCHUNK, HE], BF16, tag="E")
            nc.scalar.activation(e_t[:, 0:2, :], pss[:, 0:2 * HE], EXP)
            nc.scalar.activation(e_t[:, 2:4, :], pss[:, 2 * HE:], EXP)

            v = [sb.tile([P, NH], BF16, tag=f"v{i}", name=f"v{i}")
                 for i in range(SCHUNK)]
            nc.gpsimd.tensor_copy(v[0][:], pv[0][:])      # Pool: v0, v3
            attnT = {}
            for g in range(4):
                for sc in range(SCHUNK):
                    attnT[(g, sc)] = sb.tile([P, 4, HE], BF16,
                                             tag=f"attnT{g}_{sc}",
                                             name=f"attnT{g}_{sc}")

            def mask_op(eng, g, sc, k):
                col = 4 * g + k
                if eng is nc.scalar:
                    eng.activation(attnT[(g, sc)][:, k, :], e_t[:, sc, :],
                                   CPY, scale=masksF[:, sc, col:col + 1])
                else:
                    eng.tensor_scalar_mul(
                        attnT[(g, sc)][:, k, :], e_t[:, sc, :],
                        masksF[:, sc, col:col + 1])

            nc.scalar.activation(v[1][:], pv[1][:], CPY)  # Act: v1, v2
            nc.gpsimd.tensor_copy(v[3][:], pv[3][:])
            nc.scalar.activation(v[2][:], pv[2][:], CPY)

            # DVE: g0, g1 masks (recip slotted in after g0/sc1)
            for sc in range(SCHUNK):
                for k in range(4):
                    mask_op(nc.vector, 0, sc, k)

            # ---------- denominators (PE) + srec chain --------------------
            pS = ps.tile([EN, HE], F32, tag="small", bufs=1, name="pS")
            for sc in range(SCHUNK):
                nc.tensor.matmul(pS[:], masksB[:, sc, :], e_t[:, sc, :],
                                 start=(sc == 0), stop=(sc == SCHUNK - 1))
            srec = sb.tile([EN, HE], BF16, tag="srec")
            with nc.allow_low_precision(reason="bf16 softmax denominators"):
                nc.vector.reciprocal(srec[:], pS[:])
            srec_dram = dram.tile([EN, HE], BF16)
            nc.sync.dma_start(srec_dram[:], srec[:])
            srec_bc = []
            for g in range(4):
                t = sb.tile([P, 4, HE], BF16, tag=f"srecbc{g}",
                            name=f"srecbc{g}")
                sd = srec_dram[g * 4:(g + 1) * 4, :]
                nc.sync.dma_start(
                    t[:], bass.AP(tensor=sd.tensor, offset=sd.offset,
                                  ap=[[0, P], *sd.ap]))
                srec_bc.append(t)

            # rest of the DVE masks: g1, g2, then g3 k2/k3 (g3 k0/k1 on Act)
            for g in (1, 2):
                for sc in range(SCHUNK):
                    for k in range(4):
                        mask_op(nc.vector, g, sc, k)
            for sc in range(SCHUNK):
                mask_op(nc.vector, 3, sc, 2)
                mask_op(nc.vector, 3, sc, 3)

            # ---------- PV: po = V.T @ attnT, dc-pairs share a bank -------
            po = {}

            def pv_block(g, half):
                key = (g, half)
                po[key] = ps.tile([P, 2, 4 * NE], F32, tag="po", bufs=3,
                                  name=f"po{g}_{half}")
                for sc in range(SCHUNK):
                    for j in range(2):
                        dc = 2 * half + j
                        nc.tensor.matmul(
                            po[key][:, j, :],
                            v[sc][:, dc * P:(dc + 1) * P],
                            attnT[(g, sc)][:, :, half * NE:(half + 1) * NE],
                            start=(sc == 0), stop=(sc == SCHUNK - 1))

            for g in range(4):
                pv_block(g, 0)
                pv_block(g, 1)

            # Act: g3 k0/k1 masks sandwiched between its po copies
            outT_u = [sb.tile([P, NCHUNK, 4, NE], BF16, tag=f"outTu{g}",
                              name=f"outTu{g}") for g in range(4)]
            outT = [sb.tile([P, NCHUNK, 4, NE], BF16, tag=f"outT{g}",
                            name=f"outT{g}") for g in range(4)]

            def po_copy(eng, g, half):
                dst = outT_u[g][:, 2 * half:2 * half + 2, :, :]
                if eng is nc.scalar:
                    eng.activation(dst, po[(g, half)][:], CPY)
                else:
                    eng.tensor_copy(dst, po[(g, half)][:])

            po_copy(nc.scalar, 0, 0)
            po_copy(nc.scalar, 0, 1)
            for sc in range(SCHUNK):
                mask_op(nc.scalar, 3, sc, 0)
                mask_op(nc.scalar, 3, sc, 1)
            po_copy(nc.gpsimd, 1, 0)
            po_copy(nc.gpsimd, 1, 1)
            po_copy(nc.scalar, 2, 0)
            po_copy(nc.scalar, 2, 1)
            po_copy(nc.gpsimd, 3, 0)
            po_copy(nc.gpsimd, 3, 1)

            # ---------- normalize (DVE, bf16 2x) --------------------------
            for g in range(4):
                for half in range(2):
                    sl = srec_bc[g][:, :, half * NE:(half + 1) * NE]
                    bc = bass.AP(tensor=sl.tensor, offset=sl.offset,
                                 ap=[sl.ap[0], [0, 2], *sl.ap[1:]])
                    nc.vector.tensor_mul(
                        outT[g][:, 2 * half:2 * half + 2, :, :],
                        outT_u[g][:, 2 * half:2 * half + 2, :, :], bc)

            # ---------- O projection + output -----------------------------
            for g in range(4):
                for lp in range(2):
                    pair = g * 2 + lp
                    pO = ps.tile([P, NH], F32, tag="pv", bufs=4,
                                 name=f"pO{pair}")
                    for hc in range(NCHUNK):
                        nc.tensor.matmul(
                            pO[:], outT[g][:, hc, 2 * lp:2 * lp + 2, :],
                            wo_t[:, hc, :],
                            start=(hc == 0), stop=(hc == NCHUNK - 1))
                    o_sb = sb.tile([P, NH], BF16, tag=f"osb{pair}",
                                   name=f"osb{pair}")
                    if pair == 7:
                        nc.scalar.activation(o_sb[:, :NH // 2],
                                             pO[:, :NH // 2], CPY)
                        nc.vector.tensor_copy(o_sb[:, NH // 2:],
                                              pO[:, NH // 2:])
                    elif pair % 2 == 0:
                        nc.gpsimd.tensor_copy(o_sb[:], pO[:])
                    else:
                        nc.vector.tensor_copy(o_sb[:], pO[:])
                    nc.sync.dma_start(out_d[pair * P:(pair + 1) * P, :],
                                      o_sb[:])

    nc.compile()
    return nc


def _get_nc():
    if "nc" not in _CACHE:
        _CACHE["nc"] = _build()
    return _CACHE["nc"]


def _fast_run(nc, in_maps):
    """Repeat-call path: cached jitted shard_map over the bass PJRT primitive."""
    import jax
    from jax.sharding import Mesh, PartitionSpec
    from jax.experimental.shard_map import shard_map
    import concourse.mybir as mybir_
    from concourse import bass2jax

    if "runner" not in _CACHE:
        bass2jax.install_neuronx_cc_hook()
        part_name = (nc.partition_id_tensor.name
                     if nc.partition_id_tensor else None)
        in_names, out_names, out_avals = [], [], []
        for alloc in nc.m.functions[0].allocations:
            if not isinstance(alloc, mybir_.MemoryLocationSet):
                continue
            name = alloc.memorylocations[0].name
            if alloc.kind == "ExternalInput":
                if name != part_name:
                    in_names.append(name)
            elif alloc.kind == "ExternalOutput":
                out_names.append(name)
                out_avals.append(jax.core.ShapedArray(
                    tuple(alloc.tensor_shape), mybir_.dt.np(alloc.dtype)))
        n_params = len(in_names)
        all_in_names = in_names + out_names
        if part_name is not None:
            all_in_names = all_in_names + [part_name]

        def _body(*args):
            operands = list(args)
            if part_name is not None:
                operands.append(bass2jax.partition_id_tensor())
            outs = bass2jax._bass_exec_p.bind(
                *operands,
                out_avals=tuple(out_avals),
                in_names=tuple(all_in_names),
                out_names=tuple(out_names),
                lowering_input_output_aliases=(),
                sim_require_finite=True,
                sim_require_nnan=True,
                nc=nc,
            )
            return tuple(outs)

        devices = jax.devices()[:NB]
        mesh = Mesh(np.asarray(devices), ("core",))
        n_outs = len(out_names)
        sharded = jax.jit(
            shard_map(_body, mesh=mesh,
                      in_specs=(PartitionSpec("core"),) * (n_params + n_outs),
                      out_specs=(PartitionSpec("core"),) * n_outs,
                      check_rep=False),
            donate_argnums=tuple(range(n_params, n_params + n_outs)),
            keep_unused=True,
        )
        _CACHE["runner"] = (sharded, in_names, out_names, out_avals)

    sharded, in_names, out_names, out_avals = _CACHE["runner"]
    concat_in = [
        np.concatenate([np.asarray(m[name]) for m in in_maps], axis=0)
        for name in in_names
    ]
    concat_zeros = [
        np.zeros((NB * av.shape[0], *av.shape[1:]), av.dtype)
        for av in out_avals
    ]
    out_arrs = sharded(*concat_in, *concat_zeros)
    return [
        {name: np.asarray(out_arrs[i]).reshape(NB, *out_avals[i].shape)[c]
         for i, name in enumerate(out_names)}
        for c in range(NB)
    ]


def kernel(tokens_embed, entities, events_embed, entity_num, entity_masks,
           select_event, Wq, Wk, Wv, bq, bk, bv, Wo, bo):
    tokens_embed = np.asarray(tokens_embed, dtype=np.float32)
    entities = np.asarray(entities)
    events_embed = np.asarray(events_embed, dtype=np.float32)
    entity_masks = np.asarray(entity_masks)
    select_event = np.asarray(select_event)
    Wq = np.asarray(Wq, dtype=np.float32)
    Wk = np.asarray(Wk, dtype=np.float32)
    Wv = np.asarray(Wv, dtype=np.float32)
    Wo = np.asarray(Wo, dtype=np.float32)
    bq = np.asarray(bq, dtype=np.float32)
    bk = np.asarray(bk, dtype=np.float32)
    bv = np.asarray(bv, dtype=np.float32)
    bo = np.asarray(bo, dtype=np.float32)

    nc = _get_nc()

    q_s = (events_embed @ Wq.T + bq) * SCALE          # [NE, NH]
    # fold K projection into the query side (bk cancels in softmax):
    wtil = np.empty((NH, HE), dtype=np.float32)
    for h in range(HEADS):
        hs = slice(h * DH, (h + 1) * DH)
        wtil[:, h * NE:(h + 1) * NE] = (q_s[:, hs] @ Wk[hs, :]).T
    wtil_r = wtil.reshape(NCHUNK, P, HE)
    wv_r = np.ascontiguousarray(Wv.T).reshape(NCHUNK, P, NH)
    wo_r = np.ascontiguousarray(Wo.T).reshape(NCHUNK, P, NH)
    wo_pc = np.ascontiguousarray(
        wo_r.transpose(1, 0, 2).reshape(P, NCHUNK * NH)).astype(NPBF16)
    # attn rows sum to 1 -> bv contributes bv @ Wo.T; applied host-side.
    bo2 = (bo + bv @ Wo.T).astype(np.float32)

    shared = {"wo": wo_pc}
    in_maps = []
    for c in range(NB):
        toks_r = np.ascontiguousarray(tokens_embed[c].T).reshape(NCHUNK, P, SL)
        s0 = np.concatenate([toks_r[0], wtil_r[0]], axis=1).astype(NPBF16)
        streams = {
            "s0": np.ascontiguousarray(s0),
            "wv0": np.ascontiguousarray(wv_r[0]).astype(NPBF16),
        }
        for k in (1, 2, 3):
            sk = np.concatenate([toks_r[k], wtil_r[k], wv_r[k]],
                                axis=1).astype(NPBF16)
            streams[f"s{k}"] = np.ascontiguousarray(sk)
        # masks[p, sc*16+ent] = entities[c, ent, sc*128 + p] (f32 + bf16 view)
        m = entities[c].astype(np.float32)            # [EN, SL]
        mT = np.ascontiguousarray(
            m.reshape(EN, SCHUNK, P).transpose(2, 1, 0).reshape(P, 64))
        mpack = np.zeros((P, 96), dtype=np.float32)
        mpack[:, 0:64] = mT
        mpack[:, 64:96] = np.ascontiguousarray(
            mT.astype(NPBF16)).view(np.float32)
        streams["masks"] = mpack
        in_maps.append({**streams, **shared})

    if "ran_once" not in _CACHE:
        res = run_bass_kernel_spmd(nc, in_maps, core_ids=list(range(NB)))
        results = res.results
        _CACHE["ran_once"] = True
    else:
        results = _fast_run(nc, in_maps)
    full = np.concatenate(
        [results[c]["out"].astype(np.float32) for c in range(NB)], axis=0)
    full += bo2[None, :]

    # ragged selection (mirrors the reference indexing)
    assert int(entity_num) == EN
    entity_index = np.flatnonzero(entity_masks.reshape(-1))
    pair_sel = (select_event[:, None, :] & entity_masks[:, :, None])
    pair_sel = pair_sel.reshape(-1, NE)[entity_index].reshape(-1)
    event_entity_index = np.flatnonzero(pair_sel)

    sel_rows = (entity_index[:, None] * NE + np.arange(NE)[None, :]).reshape(-1)
    return full[sel_rows][event_entity_index]
